# revision 67
# baseline (speedup 1.0000x reference)
"""Trainium2 Bass kernel for AttentiveRelationalModuleUniformObs (v2).

Math (per sample b over N=256 neighbors, D=64, LAT=128, EC=32):
    feat   = relu(nbr @ Wf + bf)            [N, LAT]
    enc    = relu(nbr @ Wc + bc)            [N, EC]
    att    = softmax_N(cat @ Wa + ba)       [N, LAT]
    out[b] = relu((att * feat).sum(N) @ Wl + bl)

Softmax over N is shift-invariant per (b, k), so only the enc_comm block
Wa2 = Wa[EC:2EC] survives; local_data is unused.

v2 design notes (all engine budgets per group of 4 samples):
  - Everything bf16 on the PE (1 col/cycle, halves HBM traffic vs f32).
  - Transposed layout [k on partitions, n free]: per group one
    [128, 2, 256] nb tile = 4 samples (2 stacked per partition block).
  - comm encoder C[128,256]: 2 matmuls with M-zero-padded block-diagonal
    stationaries (wc_lo -> partitions 0:64, wc_hi -> 64:128, PSUM
    accumulated) so all 4 samples' EC stack the full partition dim.
  - enc = relu(C + bc) on ACT (bias per-partition), bf16 out.
  - att logits A[128,4,256]: 4 matmuls, lhsT = Wa2 zero-padded at row
    block 32j (K=128 over the 4-sample EC stack).
  - exp on ACT: ONE [128,1024] instr A->E bf16 (ACT per-instr overhead
    is ~185ns, so big instrs matter). den/num NOT via accum_out (that
    splits exp into 4 instrs + 187ns accum reads).
  - num via DVE scalar_tensor_tensor per sample:
        (F max -bf) * E, accum_out -> numSTT
    using max(F,-bf) = relu(F+bf) - bf, so
        numSTT = num_true - bf*den  =>  agg = numSTT/den + bf
    The uniform +bf is folded into the final bias row on the HOST
    (bl' = bl + bf @ Wl), so the finale is just reciprocal + one
    tensor_mul. No feat bias matmuls, no relu-feat materialization.
    (GPSIMD has no PSUM port, and two-tensor STT forms never get DVE
    fast modes, so the PSUM-reading STTs sit on DVE at ~392ns each —
    the binding engine.)
  - sample j3: ACT (which has slack) evicts relu(F+bf) to bf16 SBUF and
    that STT becomes (FRq - bf)*E — same accumulated form, SBUF-read.
  - den: the HW verifier rejects every accumulating op on Pool
    (TensorScalarPtr* are DVE-only), so Pool contributes plain
    TensorTensor halving folds of E (256 -> 64 cols) and DVE finishes
    with per-sample tensor_scalar+accum_out on the quarter tiles
    (single-tensor tensor_scalar DOES hit the 4x all-bf16 mode, 77ns).
  - Engine budgets per group: ACT ~1.84us, DVE ~1.84us, Pool ~1.71us,
    PE ~1.1us; nb DMA per PAIR of groups (HWDGE serializes ~625ns/DMA).
  - Software pipeline (stages split across iterations) so each engine
    runs back-to-back; PSUM = C(1) + A(2x2) + Fa(2) + Fb(1) = 8 banks.
    num order [j2,j0,j1,j3]: j2 first frees single-buffered Fb early;
    j3 (frq-dependent, produced late in ACT's prior iteration) goes last
    for sem margin — this killed a 23ns/group DVE gap.
  - Startup: dummy exp preloads the ACT table; w1 (wc+consts+wa, with
    f32 consts smuggled as raw bf16 halves and bitcast on-chip) lands
    before the first nb DMA; w2 (wf/wl/bl') follows; the first two nb
    loads are single-group so comm(0) starts ~0.4us earlier.
"""

import numpy as np

B, N, D, LAT, EC = 1024, 256, 64, 128, 32
M = 8           # cores
S = B // M      # samples per core (128)
G = S // 4      # main-loop groups per core (4 samples each)

_CACHE = {}


def _build_bass():
    import concourse.bacc as bacc
    import concourse.tile as tile
    from concourse import mybir

    f32 = mybir.dt.float32
    bf16 = mybir.dt.bfloat16
    AF = mybir.ActivationFunctionType
    ALU = mybir.AluOpType

    nc = bacc.Bacc("TRN2", target_bir_lowering=False)

    # DRAM I/O
    nbrT_d = nc.dram_tensor("nbrT", [S * D, N], bf16, kind="ExternalInput")
    # Weights split into two tensors so the first (comm + att + consts,
    # needed earliest) lands before the nb DMAs and the second (feat/finale)
    # can follow — every DMA serializes ~625ns on the single HWDGE.
    # w1 cols: 0:128 wc_lo | 128:256 wc_hi | 256:262 f32 consts as raw
    #          bf16 halves (bc4, -bf, +bf) | 262:774 wa4[j]
    w1_d = nc.dram_tensor("w1", [128, 774], bf16, kind="ExternalInput")
    # w2 cols: 0:128 wfh0 | 128:256 wfh1 | 256:384 wl | 384:512 bl (row 0)
    w2_d = nc.dram_tensor("w2", [128, 512], bf16, kind="ExternalInput")
    out_d = nc.dram_tensor("out", [S, LAT], f32, kind="ExternalOutput")

    with tile.TileContext(nc) as tc:
        from contextlib import ExitStack

        with ExitStack() as ctx:
            singles = ctx.enter_context(tc.tile_pool(name="singles", bufs=1))

            # dummy activation: pulls the exp/relu ACT table load off the
            # critical path (it otherwise serializes behind the weight DMA)
            dmy = singles.tile([1, 2], f32)
            nc.vector.memset(dmy, 0.0)
            dmy2 = singles.tile([1, 2], f32)
            nc.scalar.activation(out=dmy2, in_=dmy, func=AF.Exp)

            w1 = singles.tile([128, 774], bf16)
            nc.sync.dma_start(out=w1, in_=w1_d[:, :])
            w2 = singles.tile([128, 512], bf16)

            wc_lo = w1[:, 0:128]
            wc_hi = w1[:, 128:256]
            cons = w1[:, 256:262].bitcast(f32)
            wa4 = [w1[:, 262 + 128 * j : 390 + 128 * j] for j in range(4)]
            wfh = [w2[:, 0:128], w2[:, 128:256]]
            wl_sb = w2[:, 256:384]
            blr = w2[0:1, 384:512]
            bc4 = cons[:, 0:1]
            negbf = cons[:, 1:2]
            posbf = cons[:, 2:3]
            ones1 = singles.tile([1, LAT], bf16)
            nc.vector.memset(ones1, 1.0)

            num_blk = singles.tile([LAT, S], f32)
            # f32 accum_out is [p,1] per instr — exempt from the all-2-byte
            # fast-mode operand rule
            den_blk = singles.tile([LAT, S], f32)

            with ExitStack() as lctx:
                nb_pool = lctx.enter_context(tc.tile_pool(name="nb", bufs=5))
                enc_pool = lctx.enter_context(tc.tile_pool(name="enc", bufs=6))
                e_pool = lctx.enter_context(tc.tile_pool(name="e", bufs=6))
                jn_pool = lctx.enter_context(tc.tile_pool(name="jn", bufs=6))
                frq_pool = lctx.enter_context(tc.tile_pool(name="frq", bufs=5))
                eh_pool = lctx.enter_context(tc.tile_pool(name="eh", bufs=6))
                eq_pool = lctx.enter_context(tc.tile_pool(name="eq", bufs=6))
                jq_pool = lctx.enter_context(tc.tile_pool(name="jq", bufs=6))
                c_ps = lctx.enter_context(
                    tc.tile_pool(name="c_ps", bufs=1, space="PSUM")
                )
                a_ps = lctx.enter_context(
                    tc.tile_pool(name="a_ps", bufs=2, space="PSUM")
                )
                fa_ps = lctx.enter_context(
                    tc.tile_pool(name="fa_ps", bufs=2, space="PSUM")
                )
                fb_ps = lctx.enter_context(
                    tc.tile_pool(name="fb_ps", bufs=1, space="PSUM")
                )

                nbrT_ap = nbrT_d[:, :]
                nb_t = {}
                C_t = {}
                enc_t = {}
                A_t = {}
                E_t = {}
                Fa_t = {}
                Fb_t = {}
                Frq_t = {}
                Frq_t = {}

                def emit_dma(p):
                    # one DMA per PAIR of groups (8 samples) — each DMA
                    # costs ~625ns on the serialized HWDGE regardless of size
                    nb = nb_pool.tile([128, 4, N], bf16, tag="nb")
                    r0 = 8 * p * D
                    nc.sync.dma_start(
                        out=nb,
                        in_=nbrT_ap[r0 : r0 + 512, :].rearrange(
                            "(t p) n -> p t n", p=128
                        ),
                    )
                    nb_t[2 * p] = nb[:, 0:2, :]
                    nb_t[2 * p + 1] = nb[:, 2:4, :]

                def emit_dma_single(g):
                    # startup only: a single-group load halves the first
                    # transfer so comm(0) starts ~0.4us earlier
                    nb = nb_pool.tile([128, 2, N], bf16, tag=f"nbs{g}")
                    r0 = 4 * g * D
                    nc.sync.dma_start(
                        out=nb,
                        in_=nbrT_ap[r0 : r0 + 256, :].rearrange(
                            "(t p) n -> p t n", p=128
                        ),
                    )
                    nb_t[g] = nb

                def emit_comm(g):
                    C = c_ps.tile([128, N], f32, tag="C")
                    nb = nb_t[g]
                    nc.tensor.matmul(
                        out=C, lhsT=wc_lo, rhs=nb[:, 0, :], start=True, stop=False
                    )
                    nc.tensor.matmul(
                        out=C, lhsT=wc_hi, rhs=nb[:, 1, :], start=False, stop=True
                    )
                    C_t[g] = C

                def emit_enc(g):
                    enc = enc_pool.tile([128, N], bf16, tag="enc")
                    nc.scalar.activation(
                        out=enc, in_=C_t[g], func=AF.Relu, bias=bc4, scale=1.0
                    )
                    del C_t[g]
                    enc_t[g] = enc

                def emit_att(g):
                    A = a_ps.tile([128, 4, N], f32, tag="A")
                    for j in range(4):
                        nc.tensor.matmul(
                            out=A[:, j, :],
                            lhsT=wa4[j],
                            rhs=enc_t[g],
                            start=True,
                            stop=True,
                        )
                    del enc_t[g]
                    A_t[g] = A

                def emit_exp(g):
                    E = e_pool.tile([128, 4, N], bf16, tag="E")
                    nc.scalar.activation(out=E, in_=A_t[g], func=AF.Exp)
                    del A_t[g]
                    E_t[g] = E

                def emit_feat(g):
                    Fa = fa_ps.tile([128, 2, N], f32, tag="Fa")
                    Fb = fb_ps.tile([128, 2, N], f32, tag="Fb")
                    nb = nb_t[g]
                    for j in range(4):
                        t, h = j // 2, j % 2
                        Fx = Fa if j < 2 else Fb
                        nc.tensor.matmul(
                            out=Fx[:, j % 2 if j < 2 else j - 2, :],
                            lhsT=wfh[h],
                            rhs=nb[:, t, :],
                            start=True,
                            stop=True,
                        )
                    del nb_t[g]
                    Fa_t[g] = Fa
                    Fb_t[g] = Fb

                def emit_frq(g):
                    # ACT has slack: evict ONE sample (Fb half j=3) as
                    # relu(F+bf) bf16; its num STT then reads SBUF instead
                    # of PSUM (-65ns DVE) via (FRq - bf)*E = num - bf*den.
                    frq = frq_pool.tile([128, N], bf16, tag="frq")
                    nc.scalar.activation(
                        out=frq,
                        in_=Fb_t[g][:, 1, :],
                        func=AF.Relu,
                        bias=posbf,
                        scale=1.0,
                    )
                    Frq_t[g] = frq

                def emit_num(g):
                    jn = jn_pool.tile([128, 4, N], bf16, tag="jn")
                    E = E_t[g]
                    # j2,j3 first: frees single-buffered Fb early so
                    # feat(g+1) can rewrite it without stalling the PE.
                    for j in (2, 0, 1, 3):
                        s = 4 * g + j
                        if j == 3:
                            in0, op0 = Frq_t[g][:, :], ALU.add
                        else:
                            Fx = Fb_t[g] if j >= 2 else Fa_t[g]
                            in0, op0 = Fx[:, j % 2, :], ALU.max
                        nc.vector.scalar_tensor_tensor(
                            out=jn[:, j, :],
                            in0=in0,
                            scalar=negbf,
                            in1=E[:, j, :],
                            op0=op0,
                            op1=ALU.mult,
                            accum_out=num_blk[:, s : s + 1],
                        )
                    del Fa_t[g], Fb_t[g], Frq_t[g]

                def emit_den(g):
                    # Accumulating ops are DVE-only on this ISA (the HW
                    # verifier rejects TensorScalarPtr-reduce on Pool), so
                    # Pool contributes plain tensor-tensor folds of E
                    # (256 -> 64 cols per sample) and DVE finishes with one
                    # segmented all-bf16 tensor_reduce (4x perf mode).
                    E = E_t[g]
                    # plain TensorTensor is the only elementwise op the HW
                    # verifier accepts on Pool (no TensorScalarPtr variants)
                    eh = eh_pool.tile([128, 4, N // 2], bf16, tag="eh")
                    nc.gpsimd.tensor_add(
                        out=eh, in0=E[:, :, 0 : N // 2], in1=E[:, :, N // 2 : N]
                    )
                    eq = eq_pool.tile([128, 4, N // 4], bf16, tag="eq")
                    nc.gpsimd.tensor_add(
                        out=eq,
                        in0=eh[:, :, 0 : N // 4],
                        in1=eh[:, :, N // 4 : N // 2],
                    )
                    # per-sample accumulation is DVE-only on this ISA; on the
                    # quarter-folded tiles it rides the 4x all-bf16 mode
                    jq = jq_pool.tile([128, 4, N // 4], bf16, tag="jq")
                    for j in range(4):
                        s = 4 * g + j
                        nc.vector.tensor_scalar(
                            out=jq[:, j, :],
                            in0=eq[:, j, :],
                            scalar1=1.0,
                            scalar2=0.0,
                            op0=ALU.mult,
                            op1=ALU.add,
                            accum_out=den_blk[:, s : s + 1],
                        )
                    del E_t[g]

                # prologue
                emit_dma_single(0)
                nc.sync.dma_start(out=w2, in_=w2_d[:, :])
                emit_dma_single(1)
                emit_dma(1)
                emit_comm(0)
                emit_enc(0)
                emit_comm(1)
                emit_enc(1)
                emit_att(0)

                # steady-state software pipeline
                for i in range(G + 1):
                    if (i + 2) % 2 == 0 and 2 <= (i + 2) // 2 < G // 2:
                        emit_dma((i + 2) // 2)
                    if i + 2 < G:
                        emit_comm(i + 2)
                    if i + 2 < G:
                        emit_enc(i + 2)
                    if i < G:
                        emit_exp(i)
                    if i + 1 < G:
                        emit_att(i + 1)
                    if i < G:
                        emit_feat(i)
                        emit_frq(i)
                    if i >= 1:
                        emit_num(i - 1)
                        emit_den(i - 1)

            # finale: agg = numSTT/den + bf; out = relu(agg @ Wl + bl)
            with ExitStack() as fctx:
                o_ps = fctx.enter_context(
                    tc.tile_pool(name="o_ps", bufs=1, space="PSUM")
                )
                # agg = numSTT/den; the uniform +bf correction is folded
                # into the host bias row (bl' = bl + bf @ Wl).
                # reciprocal_approx_fast (199 ULP) instead of the bit-exact
                # iterative divide: ~3x fewer DVE cycles on real HW.
                rden = singles.tile([LAT, S], f32)
                nc.vector.reciprocal_approx_fast(out=rden, in_=den_blk)
                aggT = singles.tile([LAT, S], bf16)
                nc.vector.tensor_mul(out=aggT, in0=num_blk, in1=rden)

                out_ps = o_ps.tile([S, LAT], f32, tag="ops")
                nc.tensor.matmul(
                    out=out_ps, lhsT=aggT, rhs=wl_sb, start=True, stop=False
                )
                nc.tensor.matmul(
                    out=out_ps, lhsT=ones1, rhs=blr, start=False, stop=True
                )
                out_sb = singles.tile([S, LAT], f32)
                nc.vector.tensor_scalar(
                    out=out_sb,
                    in0=out_ps,
                    scalar1=0.0,
                    scalar2=None,
                    op0=ALU.max,
                )
                nc.sync.dma_start(out=out_d[:, :], in_=out_sb)

    nc.finalize()
    return nc


def _host_prep(inputs):
    import ml_dtypes

    bf16 = ml_dtypes.bfloat16

    nbr = np.ascontiguousarray(np.asarray(inputs["neighbor_data"], dtype=np.float32))
    Wf = np.asarray(inputs["Wf"], dtype=np.float32)
    bf = np.asarray(inputs["bf"], dtype=np.float32)
    Wc = np.asarray(inputs["Wc"], dtype=np.float32)
    bc = np.asarray(inputs["bc"], dtype=np.float32)
    Wa = np.asarray(inputs["Wa"], dtype=np.float32)
    Wl = np.asarray(inputs["Wl"], dtype=np.float32)
    bl = np.asarray(inputs["bl"], dtype=np.float32)

    Wa2 = Wa[EC : 2 * EC]  # only the enc_comm block survives the softmax shift

    # [M, S, N, D] -> [M, S, D, N] -> [M, S*D, N], bf16
    nbrT = (
        np.ascontiguousarray(nbr.reshape(M, S, N, D).transpose(0, 1, 3, 2))
        .reshape(M, S * D, N)
        .astype(bf16)
    )

    cons = np.zeros((128, 3), dtype="<f4")
    cons[:, 0] = np.tile(bc, 4)
    cons[:, 1] = -bf
    cons[:, 2] = bf
    # raw bf16 halves of the f32 consts (bitcast back to f32 on-chip)
    cons_raw = cons.view("<u2").view(bf16)  # [128, 6]

    w1 = np.zeros((128, 774), dtype=np.float32).astype(bf16)
    w1[0:64, 0:32] = Wc.astype(bf16)        # wc_lo -> out partitions 0:32
    w1[64:128, 32:64] = Wc.astype(bf16)     # wc_lo -> out partitions 32:64
    w1[0:64, 192:224] = Wc.astype(bf16)     # wc_hi -> out partitions 64:96
    w1[64:128, 224:256] = Wc.astype(bf16)   # wc_hi -> out partitions 96:128
    w1[:, 256:262] = cons_raw
    for j in range(4):
        w1[32 * j : 32 * j + 32, 262 + 128 * j : 390 + 128 * j] = Wa2.astype(
            bf16
        )

    w2 = np.zeros((128, 512), dtype=np.float32).astype(bf16)
    w2[0:64, 0:128] = Wf.astype(bf16)       # wfh0
    w2[64:128, 128:256] = Wf.astype(bf16)   # wfh1
    w2[:, 256:384] = Wl.astype(bf16)
    w2[0, 384:512] = (bl + bf @ Wl).astype(bf16)

    return [{"nbrT": nbrT[c], "w1": w1, "w2": w2} for c in range(M)]


def kernel(**inputs) -> np.ndarray:
    from concourse.bass_utils import run_bass_kernel_spmd

    if "nc" not in _CACHE:
        _CACHE["nc"] = _build_bass()
    nc = _CACHE["nc"]

    in_maps = _host_prep(inputs)
    res = run_bass_kernel_spmd(nc, in_maps, list(range(M)))
    out = np.concatenate(
        [np.asarray(res.results[c]["out"]) for c in range(M)], axis=0
    )
    return out.astype(np.float32)
